# revision 10
# baseline (speedup 1.0000x reference)
"""Trainium2 Bass kernel for nn_ExactLookupMerger (vq_codebook).

Strategy (8 NeuronCores, tensor-parallel over H=8192):
 - Each core owns an H-shard of 1024: constructs W1_s = W1_eff[:, hs] and
   W2_s = W2_eff[hs, :] on device (ap_gather codebook lookup + select).
 - Encode fused per batch block: h.T = c19(W1_s.T @ x.T + b1), z_part.T =
   W2_s.T @ h.T.  z partials ReduceScatter'd over batch -> each core holds
   z.T[:, batch shard] (+b2).
 - Decode reassociated: recon = z @ M + (W1_eff@db1 + db2) with
   M = W2_eff.T @ W1_eff.T  ([2048,2048], partial per core, AllReduce'd).
   Each core decodes only its batch shard: recon.T shard = M.T-ish matmuls.
 - All matmuls in fp32r (full PE rate at free dim >=256, ~1e-4 rel precision).
Host does only sharding/layout: slicing, transposes, index permutation, concat.
"""
import sys

for _p in ("/opt/trn_rl_repo",):
    if _p not in sys.path:
        sys.path.insert(0, _p)

import numpy as np
import concourse.bass as bass
import concourse.tile as tile
from concourse import bacc, mybir
from concourse.bass_utils import run_bass_kernel_spmd

F32, F32R = mybir.dt.float32, mybir.dt.float32r
I32, I16, U8 = mybir.dt.int32, mybir.dt.int16, mybir.dt.uint8
AF = mybir.ActivationFunctionType
OP = mybir.AluOpType

B, IN_D, H, OUT_D, CB = 8192, 2048, 8192, 2048, 256
NC = 8
H_S = H // NC            # 1024 H columns per core
BS = B // NC             # 1024 batch columns per core (decode shard)
NB = B // 512            # 16 batch blocks of 512
NIDX = 4096              # ap_gather indices per call (per 16-partition group)
NCALL = (IN_D * H_S) // (8 * NIDX)   # 32 gather calls per weight matrix


def _build():
    nc = bacc.Bacc("TRN2", target_bir_lowering=False, debug=False, num_devices=NC)

    # ---- inputs (per core) ----
    xT = nc.dram_tensor("xT", [IN_D, B], F32R, kind="ExternalInput")
    idx1 = nc.dram_tensor("idx1", [128, NCALL * 256], I32, kind="ExternalInput")
    idx2 = nc.dram_tensor("idx2", [128, NCALL * 256], I32, kind="ExternalInput")
    flt1 = nc.dram_tensor("flt1", [IN_D, H_S], F32, kind="ExternalInput")
    flt2 = nc.dram_tensor("flt2", [H_S, OUT_D], F32, kind="ExternalInput")
    msk1 = nc.dram_tensor("msk1", [IN_D, H_S], U8, kind="ExternalInput")
    msk2 = nc.dram_tensor("msk2", [H_S, OUT_D], U8, kind="ExternalInput")
    cb1 = nc.dram_tensor("cb1", [1, CB], F32, kind="ExternalInput")
    cb2 = nc.dram_tensor("cb2", [1, CB], F32, kind="ExternalInput")
    crw = nc.dram_tensor("crw", [128, 8], F32, kind="ExternalInput")
    rrw = nc.dram_tensor("rrw", [128, 8], F32, kind="ExternalInput")
    b1h = nc.dram_tensor("b1h", [128, 8], F32, kind="ExternalInput")
    db1h = nc.dram_tensor("db1h", [1, H_S], F32, kind="ExternalInput")
    b2h = nc.dram_tensor("b2h", [128, 16], F32, kind="ExternalInput")
    db2h = nc.dram_tensor("db2h", [128, 16], F32, kind="ExternalInput")

    # ---- outputs (per core) ----
    z_out = nc.dram_tensor("z_out", [OUT_D, BS], F32, kind="ExternalOutput")
    recon_out = nc.dram_tensor("recon_out", [IN_D, BS], F32, kind="ExternalOutput")

    with tile.TileContext(nc) as tc:
        with (
            tc.tile_pool(name="params", bufs=1) as params,
            tc.tile_pool(name="dram", bufs=1, space="DRAM") as dram,
        ):
            w1g = dram.tile([IN_D, H_S], F32, tag="w1g")
            w2g = dram.tile([H_S, OUT_D], F32, tag="w2g")
            z_stage = dram.tile([NC, OUT_D, BS], F32, tag="zst")
            zrs = dram.tile([OUT_D, BS], F32, tag="zrs")
            m_stage = dram.tile([OUT_D + 1, IN_D], F32, tag="mst")
            m_ar = dram.tile([OUT_D + 1, IN_D], F32, tag="mar")

            wctx = tc.tile_pool(name="w1pool", bufs=1)
            w1pool = wctx.__enter__()
            wctx2 = tc.tile_pool(name="w2pool", bufs=1)
            w2pool = wctx2.__enter__()
            # ---------- phase G: codebook gathers -> DRAM staging ----------
            with nc.named_scope("gather"):
                with (
                    tc.tile_pool(name="gpool", bufs=2) as gpool,
                    tc.tile_pool(name="ipool", bufs=2) as ipool,
                    tc.tile_pool(name="i16pool", bufs=2) as i16pool,
                    tc.tile_pool(name="cbpool", bufs=1) as cbpool,
                ):
                    for wsel, (cbt, idxt, stag) in enumerate(
                        [(cb1, idx1, w1g), (cb2, idx2, w2g)]
                    ):
                        cb_sb = cbpool.tile([128, CB], F32, tag=f"cb{wsel}")
                        nc.sync.dma_start(cb_sb[:], cbt.ap().to_broadcast((128, CB)))
                        stag_flat = stag.rearrange("a b -> (a b)")
                        for piece in range(NCALL // 8):
                            i32t = ipool.tile([128, 2048], I32, tag="i32")
                            nc.sync.dma_start(
                                i32t[:], idxt[:, piece * 2048:(piece + 1) * 2048]
                            )
                            i16t = i16pool.tile([128, 2048], I16, tag="i16")
                            nc.vector.tensor_copy(i16t[:], i32t[:])
                            for tt in range(8):
                                t = piece * 8 + tt
                                g_t = gpool.tile([128, NIDX], F32, tag="g")
                                nc.gpsimd.ap_gather(
                                    g_t[:], cb_sb[:],
                                    i16t[:, tt * 256:(tt + 1) * 256],
                                    channels=128, num_elems=CB, d=1, num_idxs=NIDX,
                                )
                                src = g_t.rearrange("(a b) n -> a b n", b=16)[:, 0, :]
                                dst = stag_flat[t * 8 * NIDX:(t + 1) * 8 * NIDX]
                                nc.sync.dma_start(
                                    dst.rearrange("(a b) -> a b", b=NIDX), src
                                )

            # ---------- phase S: select -> SBUF weight tiles ----------
            w1t = []
            w2t = []
            with nc.named_scope("select"):
                db1s = params.tile([128, H_S], F32, tag="db1s")
                nc.sync.dma_start(db1s[:], db1h.ap().to_broadcast((128, H_S)))
                with tc.tile_pool(name="cpool1", bufs=2) as cpool:
                    for k in range(IN_D // 128):
                        ga = cpool.tile([128, H_S], F32, tag="ga1")
                        fl = cpool.tile([128, H_S], F32, tag="fl1")
                        mk = cpool.tile([128, H_S], U8, tag="mk1")
                        nc.sync.dma_start(ga[:], w1g[k * 128:(k + 1) * 128, :])
                        nc.sync.dma_start(fl[:], flt1[k * 128:(k + 1) * 128, :])
                        nc.sync.dma_start(mk[:], msk1[k * 128:(k + 1) * 128, :])
                        wsel = cpool.tile([128, H_S], F32, tag="ws1")
                        nc.vector.select(wsel[:], mk[:], ga[:], fl[:])
                        wk = w1pool.tile([128, H_S], F32R, tag=f"w1_{k}")
                        nc.scalar.copy(wk[:], wsel[:])
                        w1t.append(wk)
                        vdum = cpool.tile([128, H_S], F32, tag="vdum")
                        vk = cpool.tile([128, 1], F32, tag="vk")
                        nc.vector.scalar_tensor_tensor(
                            vdum[:], wsel[:], 1.0,
                            db1s[:],
                            OP.mult, OP.mult, accum_out=vk[:],
                        )
                        nc.sync.dma_start(
                            m_stage[
                                OUT_D:OUT_D + 1, k * 128:(k + 1) * 128
                            ].rearrange("a b -> (a b)").rearrange(
                                "(a b) -> a b", b=1
                            ),
                            vk[:],
                        )
                with tc.tile_pool(name="cpool2", bufs=2) as cpool:
                    for k in range(H_S // 128):
                        ga = cpool.tile([128, OUT_D], F32, tag="ga2")
                        fl = cpool.tile([128, OUT_D], F32, tag="fl2")
                        mk = cpool.tile([128, OUT_D], U8, tag="mk2")
                        nc.sync.dma_start(ga[:], w2g[k * 128:(k + 1) * 128, :])
                        nc.sync.dma_start(fl[:], flt2[k * 128:(k + 1) * 128, :])
                        nc.sync.dma_start(mk[:], msk2[k * 128:(k + 1) * 128, :])
                        wsel = cpool.tile([128, OUT_D], F32, tag="ws2")
                        nc.vector.select(wsel[:], mk[:], ga[:], fl[:])
                        wk = w2pool.tile([128, OUT_D], F32R, tag=f"w2_{k}")
                        nc.scalar.copy(wk[:], wsel[:])
                        w2t.append(wk)

            # ---------- c19 per-partition params ([128, 8]) ----------
            with nc.named_scope("params"):
                craw = params.tile([128, 8], F32, tag="craw")
                rraw = params.tile([128, 8], F32, tag="rraw")
                b1s = params.tile([128, 8], F32, tag="b1s")
                b2s = params.tile([128, 16], F32, tag="b2s")
                db2s = params.tile([128, 16], F32, tag="db2s")
                nc.sync.dma_start(craw[:], crw.ap())
                nc.sync.dma_start(rraw[:], rrw.ap())
                nc.sync.dma_start(b1s[:], b1h.ap())
                nc.sync.dma_start(b2s[:], b2h.ap())
                nc.sync.dma_start(db2s[:], db2h.ap())
                c_sb = params.tile([128, 8], F32, tag="c")
                rho = params.tile([128, 8], F32, tag="rho")
                inv_c = params.tile([128, 8], F32, tag="invc")
                s1 = params.tile([128, 8], F32, tag="s1")
                s2 = params.tile([128, 8], F32, tag="s2")
                b1c = params.tile([128, 8], F32, tag="b1c")
                tmp8 = params.tile([128, 8], F32, tag="tmp8")
                exp_c = params.tile([128, 8], F32, tag="expc")
                nc.scalar.activation(exp_c[:], craw[:], AF.Exp)
                nc.scalar.activation(c_sb[:], exp_c[:], AF.Ln, bias=1.0)
                nc.scalar.activation(rho[:], rraw[:], AF.Sigmoid)
                nc.vector.reciprocal(inv_c[:], c_sb[:])
                nc.vector.tensor_scalar(tmp8[:], rho[:], -1.0, 1.0, OP.mult, OP.add)
                nc.vector.tensor_tensor(s1[:], tmp8[:], c_sb[:], OP.mult)
                nc.vector.tensor_tensor(s2[:], rho[:], b1s[:], OP.mult)
                nc.vector.tensor_tensor(b1c[:], b1s[:], inv_c[:], OP.mult)
                ones = params.tile([128, 128], F32, tag="ones")
                nc.vector.memset(ones[:], 1.0)
                ident_f = params.tile([128, 128], F32, tag="identf")
                nc.gpsimd.affine_select(
                    ident_f[:], ones[:], pattern=[[-1, 128]],
                    compare_op=OP.is_equal, fill=0.0, base=0, channel_multiplier=1,
                )
                ident = params.tile([128, 128], F32R, tag="ident")
                nc.scalar.copy(ident[:], ident_f[:])

            # ---------- encode: 16 batch blocks of 512 ----------
            with nc.named_scope("encode"):
                with (
                    tc.tile_pool(name="xpool", bufs=2) as xpool,
                    tc.tile_pool(name="hpool", bufs=10) as hpool,
                    tc.tile_pool(name="tpool", bufs=3) as tpool,
                    tc.tile_pool(name="p1", bufs=3, space="PSUM") as p1,
                    tc.tile_pool(name="p2", bufs=2, space="PSUM") as p2,
                    tc.tile_pool(name="zpool", bufs=3) as zpool,
                ):
                    for n in range(NB):
                        xh = []
                        for half in range(2):
                            xt = xpool.tile([128, 8 * 512], F32R, tag="x")
                            src = xT[
                                half * 1024:(half + 1) * 1024,
                                n * 512:(n + 1) * 512,
                            ].rearrange("(c p) j -> p c j", p=128)
                            nc.sync.dma_start(
                                xt.rearrange("p (c j) -> p c j", j=512), src
                            )
                            xh.append(xt)
                        h_tiles = []
                        for m in range(8):
                            ps = p1.tile([128, 512], F32, tag="ps1")
                            for k in range(16):
                                nc.tensor.matmul(
                                    ps[:],
                                    w1t[k][:, m * 128:(m + 1) * 128],
                                    xh[k // 8][:, (k % 8) * 512:(k % 8 + 1) * 512],
                                    start=(k == 0), stop=(k == 15),
                                )
                            t_t = tpool.tile([128, 512], F32, tag="t")
                            nc.scalar.activation(
                                t_t[:], ps[:], AF.Tanh,
                                bias=b1c[:, m:m + 1], scale=inv_c[:, m:m + 1],
                            )
                            nc.vector.tensor_scalar(
                                t_t[:], t_t[:], s1[:, m:m + 1], s2[:, m:m + 1],
                                OP.mult, OP.add,
                            )
                            h_m = hpool.tile([128, 512], F32R, tag="h")
                            nc.vector.scalar_tensor_tensor(
                                h_m[:], ps[:], rho[:, m:m + 1], t_t[:],
                                OP.mult, OP.add,
                            )
                            h_tiles.append(h_m)
                        for mo in range(16):
                            ps2 = p2.tile([128, 512], F32, tag="ps2")
                            for c in range(8):
                                nc.tensor.matmul(
                                    ps2[:],
                                    w2t[c][:, mo * 128:(mo + 1) * 128],
                                    h_tiles[c][:],
                                    start=(c == 0), stop=(c == 7),
                                )
                            zt = zpool.tile([128, 512], F32, tag="z")
                            nc.scalar.copy(zt[:], ps2[:])
                            nc.sync.dma_start(
                                z_stage[
                                    n // 2,
                                    mo * 128:(mo + 1) * 128,
                                    (n % 2) * 512:(n % 2 + 1) * 512,
                                ],
                                zt[:],
                            )

            # ---------- ReduceScatter z over batch ----------
            with nc.named_scope("rs"):
                nc.gpsimd.collective_compute(
                    "ReduceScatter", OP.add,
                    replica_groups=[list(range(NC))],
                    ins=[z_stage.opt()], outs=[zrs.opt()],
                )

            # ---------- M = W2_eff.T @ W1_eff.T partial + v = W1@db1 ----------
            with nc.named_scope("mbuild"):
                with (
                    tc.tile_pool(name="w1tp", bufs=1) as w1tp,
                    tc.tile_pool(name="pt", bufs=2, space="PSUM") as pt,
                    tc.tile_pool(name="pm", bufs=2, space="PSUM") as pm,
                    tc.tile_pool(name="mout", bufs=3) as mout,
                ):
                    for half in range(2):
                        # transpose W1_s chunks of this in-half -> W1T half tiles
                        ht_tiles = []
                        for c in range(8):
                            w1tc = w1tp.tile([128, 1024], F32R, tag=f"w1t_{c}")
                            for kk in range(8):
                                k = half * 8 + kk
                                pst = pt.tile([128, 128], F32R, tag="pst")
                                nc.tensor.transpose(
                                    pst[:],
                                    w1t[k][:, c * 128:(c + 1) * 128],
                                    ident[:],
                                )
                                nc.scalar.copy(
                                    w1tc[:, kk * 128:(kk + 1) * 128], pst[:]
                                )
                            ht_tiles.append(w1tc)
                        for mo in range(16):
                            for nin in range(2):
                                psm = pm.tile([128, 512], F32, tag="psm")
                                for c in range(8):
                                    nc.tensor.matmul(
                                        psm[:],
                                        w2t[c][:, mo * 128:(mo + 1) * 128],
                                        ht_tiles[c][:, nin * 512:(nin + 1) * 512],
                                        start=(c == 0), stop=(c == 7),
                                    )
                                ms = mout.tile([128, 512], F32, tag="ms")
                                nc.scalar.copy(ms[:], psm[:])
                                nc.sync.dma_start(
                                    m_stage[
                                        mo * 128:(mo + 1) * 128,
                                        half * 1024 + nin * 512:
                                        half * 1024 + (nin + 1) * 512,
                                    ],
                                    ms[:],
                                )

            with nc.named_scope("arm"):
                nc.gpsimd.collective_compute(
                    "AllReduce", OP.add,
                    replica_groups=[list(range(NC))],
                    ins=[m_stage.opt()], outs=[m_ar.opt()],
                )

            wctx2.__exit__(None, None, None)
            wctx.__exit__(None, None, None)

            # ---------- decode: recon.T shard = sum_k M[k,:].T @ z.T[k,:] ----------
            with nc.named_scope("decode"):
                with (
                    tc.tile_pool(name="mpool", bufs=1) as mpool,
                    tc.tile_pool(name="zq", bufs=3) as zq,
                    tc.tile_pool(name="zr", bufs=17) as zr,
                    tc.tile_pool(name="p3", bufs=3, space="PSUM") as p3,
                    tc.tile_pool(name="ro", bufs=3) as ro,
                ):
                    vd = params.tile([128, 16], F32, tag="vd")
                    nc.sync.dma_start(
                        vd[:],
                        m_ar[OUT_D:OUT_D + 1, :].rearrange(
                            "one (m p) -> (one p) m", p=128
                        ),
                    )
                    nc.vector.tensor_tensor(vd[:], vd[:], db2s[:], OP.add)
                    m_tiles = []
                    for k in range(16):
                        mt = mpool.tile([128, IN_D], F32R, tag=f"m_{k}")
                        nc.sync.dma_start(
                            mt[:], m_ar[k * 128:(k + 1) * 128, :].bitcast(F32R)
                        )
                        m_tiles.append(mt)
                    for n in range(2):
                        zr_tiles = []
                        for k in range(16):
                            zt = zq.tile([128, 512], F32, tag="zq")
                            nc.sync.dma_start(
                                zt[:],
                                zrs[k * 128:(k + 1) * 128, n * 512:(n + 1) * 512],
                            )
                            nc.vector.tensor_scalar(
                                zt[:], zt[:], b2s[:, k:k + 1], None, OP.add
                            )
                            nc.sync.dma_start(
                                z_out[k * 128:(k + 1) * 128, n * 512:(n + 1) * 512],
                                zt[:],
                            )
                            zk = zr.tile([128, 512], F32R, tag="zr")
                            nc.scalar.copy(zk[:], zt[:])
                            zr_tiles.append(zk)
                        for m in range(16):
                            ps3 = p3.tile([128, 512], F32, tag="ps3")
                            for k in range(16):
                                nc.tensor.matmul(
                                    ps3[:],
                                    m_tiles[k][:, m * 128:(m + 1) * 128],
                                    zr_tiles[k][:],
                                    start=(k == 0), stop=(k == 15),
                                )
                            rt = ro.tile([128, 512], F32, tag="ro")
                            nc.vector.tensor_scalar(
                                rt[:], ps3[:], vd[:, m:m + 1], None, OP.add
                            )
                            nc.sync.dma_start(
                                recon_out[
                                    m * 128:(m + 1) * 128, n * 512:(n + 1) * 512
                                ],
                                rt[:],
                            )

    nc.compile()
    return nc


_CACHE = {}


def _get_nc():
    if "nc" not in _CACHE:
        _CACHE["nc"] = _build()
    return _CACHE["nc"]


def _perm_idx(idx_s):
    """[2048, 1024] int32 -> [128, NCALL*256] ap_gather stream layout."""
    v = idx_s.reshape(NCALL, 8, 4, 1024).reshape(NCALL, 8, NIDX)
    v = v.reshape(NCALL, 8, 256, 16)
    return np.ascontiguousarray(
        v.transpose(1, 3, 0, 2).reshape(128, NCALL * 256)
    ).astype(np.int32)


def _prep_in_maps(inputs):
    x = np.asarray(inputs["x"], np.float32)
    xT = np.ascontiguousarray(x.T)
    cb1 = np.asarray(inputs["codebook_W1"], np.float32).reshape(1, CB)
    cb2 = np.asarray(inputs["codebook_W2"], np.float32).reshape(1, CB)
    W1f, W2f = np.asarray(inputs["W1_float"], np.float32), np.asarray(inputs["W2_float"], np.float32)
    W1i, W2i = np.asarray(inputs["W1_idx"], np.int32), np.asarray(inputs["W2_idx"], np.int32)
    W1m = np.asarray(inputs["W1_frozen_mask"]).astype(np.uint8)
    W2m = np.asarray(inputs["W2_frozen_mask"]).astype(np.uint8)
    b1 = np.asarray(inputs["b1"], np.float32)
    b2 = np.asarray(inputs["b2"], np.float32)
    db1 = np.asarray(inputs["db1"], np.float32)
    db2 = np.asarray(inputs["db2"], np.float32)
    craw = np.asarray(inputs["c19_c_raw"], np.float32)
    rraw = np.asarray(inputs["c19_rho_raw"], np.float32)

    def p8(v):   # [1024] -> [128, 8]
        return np.ascontiguousarray(v.reshape(8, 128).T)

    def p16(v):  # [2048] -> [128, 16]
        return np.ascontiguousarray(v.reshape(16, 128).T)

    in_maps = []
    for c in range(NC):
        hs = slice(H_S * c, H_S * (c + 1))
        in_maps.append(dict(
            xT=xT,
            idx1=_perm_idx(np.ascontiguousarray(W1i[:, hs])),
            idx2=_perm_idx(np.ascontiguousarray(W2i[hs, :]).reshape(IN_D, H_S)),
            flt1=np.ascontiguousarray(W1f[:, hs]),
            flt2=np.ascontiguousarray(W2f[hs, :]),
            msk1=np.ascontiguousarray(W1m[:, hs]),
            msk2=np.ascontiguousarray(W2m[hs, :]),
            cb1=cb1, cb2=cb2,
            crw=p8(craw[hs]), rrw=p8(rraw[hs]), b1h=p8(b1[hs]),
            db1h=np.ascontiguousarray(db1[hs]).reshape(1, H_S), b2h=p16(b2), db2h=p16(db2),
        ))
    return in_maps


def _assemble(results):
    reconT = np.concatenate([results[c]["recon_out"] for c in range(NC)], axis=1)
    zT = np.concatenate([results[c]["z_out"] for c in range(NC)], axis=1)
    recon = np.ascontiguousarray(reconT.T, dtype=np.float32)
    z = np.ascontiguousarray(zT.T, dtype=np.float32)
    return recon, z


def kernel(**inputs):
    nc = _get_nc()
    in_maps = _prep_in_maps(inputs)
    res = run_bass_kernel_spmd(nc, in_maps, core_ids=list(range(NC)))
    return _assemble(res.results)


# revision 11
# speedup vs baseline: 1.0094x; 1.0094x over previous
"""Trainium2 Bass kernel for nn_ExactLookupMerger (vq_codebook).

Strategy (8 NeuronCores, tensor-parallel over H=8192):
 - Each core owns an H-shard of 1024: constructs W1_s = W1_eff[:, hs] and
   W2_s = W2_eff[hs, :] on device (ap_gather codebook lookup + select).
 - Encode fused per batch block: h.T = c19(W1_s.T @ x.T + b1), z_part.T =
   W2_s.T @ h.T.  z partials ReduceScatter'd over batch -> each core holds
   z.T[:, batch shard] (+b2).
 - Decode reassociated: recon = z @ M + (W1_eff@db1 + db2) with
   M = W2_eff.T @ W1_eff.T  ([2048,2048], partial per core, AllReduce'd).
   Each core decodes only its batch shard: recon.T shard = M.T-ish matmuls.
 - All matmuls in fp32r (full PE rate at free dim >=256, ~1e-4 rel precision).
Host does only sharding/layout: slicing, transposes, index permutation, concat.
"""
import sys

for _p in ("/opt/trn_rl_repo",):
    if _p not in sys.path:
        sys.path.insert(0, _p)

import numpy as np
import concourse.bass as bass
import concourse.tile as tile
from concourse import bacc, mybir
from concourse.bass_utils import run_bass_kernel_spmd

F32, F32R = mybir.dt.float32, mybir.dt.float32r
I32, I16, U8 = mybir.dt.int32, mybir.dt.int16, mybir.dt.uint8
AF = mybir.ActivationFunctionType
OP = mybir.AluOpType

B, IN_D, H, OUT_D, CB = 8192, 2048, 8192, 2048, 256
NC = 8
H_S = H // NC            # 1024 H columns per core
BS = B // NC             # 1024 batch columns per core (decode shard)
NB = B // 512            # 16 batch blocks of 512
NIDX = 2048              # ap_gather indices per call (per 16-partition group)
NCALLS = (IN_D * H_S) // (8 * NIDX)  # 128 gather calls per weight matrix
IDXCOLS = NCALLS * 128               # int32 index columns per weight


def _build():
    nc = bacc.Bacc("TRN2", target_bir_lowering=False, debug=False, num_devices=NC)

    # ---- inputs (per core) ----
    xT = nc.dram_tensor("xT", [IN_D, B], F32R, kind="ExternalInput")
    idx1 = nc.dram_tensor("idx1", [128, IDXCOLS], I32, kind="ExternalInput")
    idx2 = nc.dram_tensor("idx2", [128, IDXCOLS], I32, kind="ExternalInput")
    flt1 = nc.dram_tensor("flt1", [IN_D, H_S], F32, kind="ExternalInput")
    flt2 = nc.dram_tensor("flt2", [H_S, OUT_D], F32, kind="ExternalInput")
    msk1 = nc.dram_tensor("msk1", [IN_D, H_S], U8, kind="ExternalInput")
    msk2 = nc.dram_tensor("msk2", [H_S, OUT_D], U8, kind="ExternalInput")
    cb1 = nc.dram_tensor("cb1", [1, CB], F32, kind="ExternalInput")
    cb2 = nc.dram_tensor("cb2", [1, CB], F32, kind="ExternalInput")
    crw = nc.dram_tensor("crw", [128, 8], F32, kind="ExternalInput")
    rrw = nc.dram_tensor("rrw", [128, 8], F32, kind="ExternalInput")
    b1h = nc.dram_tensor("b1h", [128, 8], F32, kind="ExternalInput")
    db1h = nc.dram_tensor("db1h", [1, H_S], F32, kind="ExternalInput")
    b2h = nc.dram_tensor("b2h", [128, 16], F32, kind="ExternalInput")
    db2h = nc.dram_tensor("db2h", [128, 16], F32, kind="ExternalInput")

    # ---- outputs (per core) ----
    z_out = nc.dram_tensor("z_out", [OUT_D, BS], F32, kind="ExternalOutput")
    recon_out = nc.dram_tensor("recon_out", [IN_D, BS], F32, kind="ExternalOutput")

    with tile.TileContext(nc) as tc:
        with (
            tc.tile_pool(name="params", bufs=1) as params,
            tc.tile_pool(name="dram", bufs=1, space="DRAM") as dram,
        ):
            z_stage = dram.tile([NC, OUT_D, BS], F32, tag="zst")
            zrs = dram.tile([OUT_D, BS], F32, tag="zrs")
            m_stage = dram.tile([OUT_D + 1, IN_D], F32, tag="mst")
            m_ar = dram.tile([OUT_D + 1, IN_D], F32, tag="mar")

            wctx = tc.tile_pool(name="w1pool", bufs=1)
            w1pool = wctx.__enter__()
            wctx2 = tc.tile_pool(name="w2pool", bufs=1)
            w2pool = wctx2.__enter__()
            # ---- construct W tiles: ap_gather + DVE-transpose extract + select ----
            w1t = []
            w2t = []
            with nc.named_scope("construct"):
                db1s = params.tile([128, H_S], F32, tag="db1s")
                nc.sync.dma_start(db1s[:], db1h.ap().to_broadcast((128, H_S)))
                with (
                    tc.tile_pool(name="gpool", bufs=2) as gpool,
                    tc.tile_pool(name="trpool", bufs=2) as trpool,
                    tc.tile_pool(name="ipool", bufs=1) as ipool,
                    tc.tile_pool(name="i16pool", bufs=1) as i16pool,
                    tc.tile_pool(name="cbpool", bufs=1) as cbpool,
                    tc.tile_pool(name="spool", bufs=1) as spool,
                    tc.tile_pool(name="gbpool", bufs=3) as gbpool,
                ):
                    for wi, (cbt, idxt, fltt, mskt, wpool_, wlist, RC, CW) in \
                            enumerate([
                                (cb1, idx1, flt1, msk1, w1pool, w1t, 16, H_S),
                                (cb2, idx2, flt2, msk2, w2pool, w2t, 8, OUT_D),
                            ]):
                        cb_sb = cbpool.tile([128, CB], F32, tag=f"cb{wi}")
                        nc.sync.dma_start(cb_sb[:], cbt.ap().to_broadcast((128, CB)))
                        cpc = CW // 128
                        i16t = None
                        for k in range(RC):
                            fl = spool.tile([128, CW], F32, tag=f"fl{wi}")
                            mk = spool.tile([128, CW], U8, tag=f"mk{wi}")
                            nc.sync.dma_start(fl[:], fltt[k * 128:(k + 1) * 128, :])
                            nc.sync.dma_start(mk[:], mskt[k * 128:(k + 1) * 128, :])
                            wk = wpool_.tile([128, CW], F32R, tag=f"w{wi}_{k}")
                            if wi == 0:
                                vk8 = spool.tile([128, cpc], F32, tag="vk8")
                            for cc in range(cpc):
                                t = k * cpc + cc
                                if t % 16 == 0:
                                    i32t = ipool.tile([128, 2048], I32, tag="i32")
                                    nc.sync.dma_start(
                                        i32t[:],
                                        idxt[:, (t // 16) * 2048:(t // 16 + 1) * 2048],
                                    )
                                    i16t = i16pool.tile([128, 2048], I16, tag="i16")
                                    nc.vector.tensor_copy(i16t[:], i32t[:])
                                g_t = gpool.tile([128, NIDX], F32, tag="g")
                                nc.gpsimd.ap_gather(
                                    g_t[:], cb_sb[:],
                                    i16t[:, (t % 16) * 128:(t % 16 + 1) * 128],
                                    channels=128, num_elems=CB, d=1, num_idxs=NIDX,
                                )
                                tr = trpool.tile([128, NIDX], F32, tag="tr")
                                nc.vector.transpose(tr[:], g_t[:])
                                view = tr.rearrange("p (f b) -> p f b", b=32)
                                gb = gbpool.tile([128, 128], F32, tag="gb")
                                nc.vector.tensor_copy(gb[:, 0:64], view[:, :, 0])
                                nc.vector.tensor_copy(gb[:, 64:128], view[:, :, 16])
                                ws = gbpool.tile([128, 128], F32, tag="wsp")
                                sl = slice(cc * 128, (cc + 1) * 128)
                                nc.vector.tensor_copy(ws[:], fl[:, sl])
                                nc.vector.copy_predicated(ws[:], mk[:, sl], gb[:])
                                nc.scalar.copy(wk[:, sl], ws[:])
                                if wi == 0:
                                    vdum = gbpool.tile([128, 128], F32, tag="vdum")
                                    nc.vector.scalar_tensor_tensor(
                                        vdum[:], ws[:], 1.0, db1s[:, sl],
                                        OP.mult, OP.mult,
                                        accum_out=vk8[:, cc:cc + 1],
                                    )
                            wlist.append(wk)
                            if wi == 0:
                                vk = spool.tile([128, 1], F32, tag="vk")
                                nc.vector.reduce_sum(
                                    vk[:], vk8[:], axis=mybir.AxisListType.X
                                )
                                nc.sync.dma_start(
                                    m_stage[
                                        OUT_D:OUT_D + 1, k * 128:(k + 1) * 128
                                    ].rearrange("a b -> (a b)").rearrange(
                                        "(a b) -> a b", b=1
                                    ),
                                    vk[:],
                                )

            # ---------- c19 per-partition params ([128, 8]) ----------
            with nc.named_scope("params"):
                craw = params.tile([128, 8], F32, tag="craw")
                rraw = params.tile([128, 8], F32, tag="rraw")
                b1s = params.tile([128, 8], F32, tag="b1s")
                b2s = params.tile([128, 16], F32, tag="b2s")
                db2s = params.tile([128, 16], F32, tag="db2s")
                nc.sync.dma_start(craw[:], crw.ap())
                nc.sync.dma_start(rraw[:], rrw.ap())
                nc.sync.dma_start(b1s[:], b1h.ap())
                nc.sync.dma_start(b2s[:], b2h.ap())
                nc.sync.dma_start(db2s[:], db2h.ap())
                c_sb = params.tile([128, 8], F32, tag="c")
                rho = params.tile([128, 8], F32, tag="rho")
                inv_c = params.tile([128, 8], F32, tag="invc")
                s1 = params.tile([128, 8], F32, tag="s1")
                s2 = params.tile([128, 8], F32, tag="s2")
                b1c = params.tile([128, 8], F32, tag="b1c")
                tmp8 = params.tile([128, 8], F32, tag="tmp8")
                exp_c = params.tile([128, 8], F32, tag="expc")
                nc.scalar.activation(exp_c[:], craw[:], AF.Exp)
                nc.scalar.activation(c_sb[:], exp_c[:], AF.Ln, bias=1.0)
                nc.scalar.activation(rho[:], rraw[:], AF.Sigmoid)
                nc.vector.reciprocal(inv_c[:], c_sb[:])
                nc.vector.tensor_scalar(tmp8[:], rho[:], -1.0, 1.0, OP.mult, OP.add)
                nc.vector.tensor_tensor(s1[:], tmp8[:], c_sb[:], OP.mult)
                nc.vector.tensor_tensor(s2[:], rho[:], b1s[:], OP.mult)
                nc.vector.tensor_tensor(b1c[:], b1s[:], inv_c[:], OP.mult)
                ones = params.tile([128, 128], F32, tag="ones")
                nc.vector.memset(ones[:], 1.0)
                ident_f = params.tile([128, 128], F32, tag="identf")
                nc.gpsimd.affine_select(
                    ident_f[:], ones[:], pattern=[[-1, 128]],
                    compare_op=OP.is_equal, fill=0.0, base=0, channel_multiplier=1,
                )
                ident = params.tile([128, 128], F32R, tag="ident")
                nc.scalar.copy(ident[:], ident_f[:])

            # ---------- encode: 16 batch blocks of 512 ----------
            with nc.named_scope("encode"):
                with (
                    tc.tile_pool(name="xpool", bufs=2) as xpool,
                    tc.tile_pool(name="hpool", bufs=10) as hpool,
                    tc.tile_pool(name="tpool", bufs=3) as tpool,
                    tc.tile_pool(name="p1", bufs=3, space="PSUM") as p1,
                    tc.tile_pool(name="p2", bufs=2, space="PSUM") as p2,
                    tc.tile_pool(name="zpool", bufs=3) as zpool,
                ):
                    for n in range(NB):
                        xh = []
                        for half in range(2):
                            xt = xpool.tile([128, 8 * 512], F32R, tag="x")
                            src = xT[
                                half * 1024:(half + 1) * 1024,
                                n * 512:(n + 1) * 512,
                            ].rearrange("(c p) j -> p c j", p=128)
                            nc.sync.dma_start(
                                xt.rearrange("p (c j) -> p c j", j=512), src
                            )
                            xh.append(xt)
                        h_tiles = []
                        for m in range(8):
                            ps = p1.tile([128, 512], F32, tag="ps1")
                            for k in range(16):
                                nc.tensor.matmul(
                                    ps[:],
                                    w1t[k][:, m * 128:(m + 1) * 128],
                                    xh[k // 8][:, (k % 8) * 512:(k % 8 + 1) * 512],
                                    start=(k == 0), stop=(k == 15),
                                )
                            t_t = tpool.tile([128, 512], F32, tag="t")
                            nc.scalar.activation(
                                t_t[:], ps[:], AF.Tanh,
                                bias=b1c[:, m:m + 1], scale=inv_c[:, m:m + 1],
                            )
                            nc.vector.tensor_scalar(
                                t_t[:], t_t[:], s1[:, m:m + 1], s2[:, m:m + 1],
                                OP.mult, OP.add,
                            )
                            h_m = hpool.tile([128, 512], F32R, tag="h")
                            nc.vector.scalar_tensor_tensor(
                                h_m[:], ps[:], rho[:, m:m + 1], t_t[:],
                                OP.mult, OP.add,
                            )
                            h_tiles.append(h_m)
                        for mo in range(16):
                            ps2 = p2.tile([128, 512], F32, tag="ps2")
                            for c in range(8):
                                nc.tensor.matmul(
                                    ps2[:],
                                    w2t[c][:, mo * 128:(mo + 1) * 128],
                                    h_tiles[c][:],
                                    start=(c == 0), stop=(c == 7),
                                )
                            zt = zpool.tile([128, 512], F32, tag="z")
                            nc.scalar.copy(zt[:], ps2[:])
                            nc.sync.dma_start(
                                z_stage[
                                    n // 2,
                                    mo * 128:(mo + 1) * 128,
                                    (n % 2) * 512:(n % 2 + 1) * 512,
                                ],
                                zt[:],
                            )

            # ---------- ReduceScatter z over batch ----------
            with nc.named_scope("rs"):
                nc.gpsimd.collective_compute(
                    "ReduceScatter", OP.add,
                    replica_groups=[list(range(NC))],
                    ins=[z_stage.opt()], outs=[zrs.opt()],
                )

            # ---------- M = W2_eff.T @ W1_eff.T partial + v = W1@db1 ----------
            with nc.named_scope("mbuild"):
                with (
                    tc.tile_pool(name="w1tp", bufs=1) as w1tp,
                    tc.tile_pool(name="pt", bufs=2, space="PSUM") as pt,
                    tc.tile_pool(name="pm", bufs=2, space="PSUM") as pm,
                    tc.tile_pool(name="mout", bufs=3) as mout,
                ):
                    for half in range(2):
                        # transpose W1_s chunks of this in-half -> W1T half tiles
                        ht_tiles = []
                        for c in range(8):
                            w1tc = w1tp.tile([128, 1024], F32R, tag=f"w1t_{c}")
                            for kk in range(8):
                                k = half * 8 + kk
                                pst = pt.tile([128, 128], F32R, tag="pst")
                                nc.tensor.transpose(
                                    pst[:],
                                    w1t[k][:, c * 128:(c + 1) * 128],
                                    ident[:],
                                )
                                nc.scalar.copy(
                                    w1tc[:, kk * 128:(kk + 1) * 128], pst[:]
                                )
                            ht_tiles.append(w1tc)
                        for mo in range(16):
                            for nin in range(2):
                                psm = pm.tile([128, 512], F32, tag="psm")
                                for c in range(8):
                                    nc.tensor.matmul(
                                        psm[:],
                                        w2t[c][:, mo * 128:(mo + 1) * 128],
                                        ht_tiles[c][:, nin * 512:(nin + 1) * 512],
                                        start=(c == 0), stop=(c == 7),
                                    )
                                ms = mout.tile([128, 512], F32, tag="ms")
                                nc.scalar.copy(ms[:], psm[:])
                                nc.sync.dma_start(
                                    m_stage[
                                        mo * 128:(mo + 1) * 128,
                                        half * 1024 + nin * 512:
                                        half * 1024 + (nin + 1) * 512,
                                    ],
                                    ms[:],
                                )

            with nc.named_scope("arm"):
                nc.gpsimd.collective_compute(
                    "AllReduce", OP.add,
                    replica_groups=[list(range(NC))],
                    ins=[m_stage.opt()], outs=[m_ar.opt()],
                )

            wctx2.__exit__(None, None, None)
            wctx.__exit__(None, None, None)

            # ---------- decode: recon.T shard = sum_k M[k,:].T @ z.T[k,:] ----------
            with nc.named_scope("decode"):
                with (
                    tc.tile_pool(name="mpool", bufs=1) as mpool,
                    tc.tile_pool(name="zq", bufs=3) as zq,
                    tc.tile_pool(name="zr", bufs=17) as zr,
                    tc.tile_pool(name="p3", bufs=3, space="PSUM") as p3,
                    tc.tile_pool(name="ro", bufs=3) as ro,
                ):
                    vd = params.tile([128, 16], F32, tag="vd")
                    nc.sync.dma_start(
                        vd[:],
                        m_ar[OUT_D:OUT_D + 1, :].rearrange(
                            "one (m p) -> (one p) m", p=128
                        ),
                    )
                    nc.vector.tensor_tensor(vd[:], vd[:], db2s[:], OP.add)
                    m_tiles = []
                    for k in range(16):
                        mt = mpool.tile([128, IN_D], F32R, tag=f"m_{k}")
                        nc.sync.dma_start(
                            mt[:], m_ar[k * 128:(k + 1) * 128, :].bitcast(F32R)
                        )
                        m_tiles.append(mt)
                    for n in range(2):
                        zr_tiles = []
                        for k in range(16):
                            zt = zq.tile([128, 512], F32, tag="zq")
                            nc.sync.dma_start(
                                zt[:],
                                zrs[k * 128:(k + 1) * 128, n * 512:(n + 1) * 512],
                            )
                            nc.vector.tensor_scalar(
                                zt[:], zt[:], b2s[:, k:k + 1], None, OP.add
                            )
                            nc.sync.dma_start(
                                z_out[k * 128:(k + 1) * 128, n * 512:(n + 1) * 512],
                                zt[:],
                            )
                            zk = zr.tile([128, 512], F32R, tag="zr")
                            nc.scalar.copy(zk[:], zt[:])
                            zr_tiles.append(zk)
                        for m in range(16):
                            ps3 = p3.tile([128, 512], F32, tag="ps3")
                            for k in range(16):
                                nc.tensor.matmul(
                                    ps3[:],
                                    m_tiles[k][:, m * 128:(m + 1) * 128],
                                    zr_tiles[k][:],
                                    start=(k == 0), stop=(k == 15),
                                )
                            rt = ro.tile([128, 512], F32, tag="ro")
                            nc.vector.tensor_scalar(
                                rt[:], ps3[:], vd[:, m:m + 1], None, OP.add
                            )
                            nc.sync.dma_start(
                                recon_out[
                                    m * 128:(m + 1) * 128, n * 512:(n + 1) * 512
                                ],
                                rt[:],
                            )

    nc.compile()
    return nc


_CACHE = {}


def _get_nc():
    if "nc" not in _CACHE:
        _CACHE["nc"] = _build()
    return _CACHE["nc"]


def _perm_idx(idx_s, RC, CW):
    """IDX chunk layout -> ap_gather stream layout [128, 16384].

    Call (k, cc) covers chunk k cols [128cc, 128cc+128). Strip g=2P+h holds,
    at element i=32F+a, IDX[128k + 32P + a, 128cc + 64h + F]; streams are
    16-wrapped per group: A[16g+q, s] = u_g[16s+q]."""
    ncalls = RC * (CW // 128)
    v = idx_s.reshape(RC, 4, 32, CW // 128, 2, 64)       # [k,P,a,cc,h,F]
    u = v.transpose(0, 3, 1, 4, 5, 2)                    # [k,cc,P,h,F,a]
    u = u.reshape(ncalls, 8, NIDX)                       # [call,g,i]
    A = u.reshape(ncalls, 8, 128, 16).transpose(0, 1, 3, 2)  # [call,g,q,s]
    A = A.reshape(ncalls, 128, 128)
    return np.ascontiguousarray(
        A.transpose(1, 0, 2).reshape(128, ncalls * 128)
    ).astype(np.int32)


def _prep_in_maps(inputs):
    x = np.asarray(inputs["x"], np.float32)
    xT = np.ascontiguousarray(x.T)
    cb1 = np.asarray(inputs["codebook_W1"], np.float32).reshape(1, CB)
    cb2 = np.asarray(inputs["codebook_W2"], np.float32).reshape(1, CB)
    W1f, W2f = np.asarray(inputs["W1_float"], np.float32), np.asarray(inputs["W2_float"], np.float32)
    W1i, W2i = np.asarray(inputs["W1_idx"], np.int32), np.asarray(inputs["W2_idx"], np.int32)
    W1m = np.asarray(inputs["W1_frozen_mask"]).astype(np.uint8)
    W2m = np.asarray(inputs["W2_frozen_mask"]).astype(np.uint8)
    b1 = np.asarray(inputs["b1"], np.float32)
    b2 = np.asarray(inputs["b2"], np.float32)
    db1 = np.asarray(inputs["db1"], np.float32)
    db2 = np.asarray(inputs["db2"], np.float32)
    craw = np.asarray(inputs["c19_c_raw"], np.float32)
    rraw = np.asarray(inputs["c19_rho_raw"], np.float32)

    def p8(v):   # [1024] -> [128, 8]
        return np.ascontiguousarray(v.reshape(8, 128).T)

    def p16(v):  # [2048] -> [128, 16]
        return np.ascontiguousarray(v.reshape(16, 128).T)

    in_maps = []
    for c in range(NC):
        hs = slice(H_S * c, H_S * (c + 1))
        in_maps.append(dict(
            xT=xT,
            idx1=_perm_idx(np.ascontiguousarray(W1i[:, hs]), 16, H_S),
            idx2=_perm_idx(np.ascontiguousarray(W2i[hs, :]), 8, OUT_D),
            flt1=np.ascontiguousarray(W1f[:, hs]),
            flt2=np.ascontiguousarray(W2f[hs, :]),
            msk1=np.ascontiguousarray(W1m[:, hs]),
            msk2=np.ascontiguousarray(W2m[hs, :]),
            cb1=cb1, cb2=cb2,
            crw=p8(craw[hs]), rrw=p8(rraw[hs]), b1h=p8(b1[hs]),
            db1h=np.ascontiguousarray(db1[hs]).reshape(1, H_S), b2h=p16(b2), db2h=p16(db2),
        ))
    return in_maps


def _assemble(results):
    reconT = np.concatenate([results[c]["recon_out"] for c in range(NC)], axis=1)
    zT = np.concatenate([results[c]["z_out"] for c in range(NC)], axis=1)
    recon = np.ascontiguousarray(reconT.T, dtype=np.float32)
    z = np.ascontiguousarray(zT.T, dtype=np.float32)
    return recon, z


def kernel(**inputs):
    nc = _get_nc()
    in_maps = _prep_in_maps(inputs)
    res = run_bass_kernel_spmd(nc, in_maps, core_ids=list(range(NC)))
    return _assemble(res.results)


# revision 12
# speedup vs baseline: 2.2818x; 2.2607x over previous
"""Trainium2 Bass kernel for nn_ExactLookupMerger (vq_codebook).

Strategy (8 NeuronCores, tensor-parallel over H=8192):
 - Each core owns an H-shard of 1024: constructs W1_s = W1_eff[:, hs] and
   W2_s = W2_eff[hs, :] on device (ap_gather codebook lookup + select).
 - Encode fused per batch block: h.T = c19(W1_s.T @ x.T + b1), z_part.T =
   W2_s.T @ h.T.  z partials ReduceScatter'd over batch -> each core holds
   z.T[:, batch shard] (+b2).
 - Decode reassociated: recon = z @ M + (W1_eff@db1 + db2) with
   M = W2_eff.T @ W1_eff.T  ([2048,2048], partial per core, AllReduce'd).
   Each core decodes only its batch shard: recon.T shard = M.T-ish matmuls.
 - All matmuls in fp32r (full PE rate at free dim >=256, ~1e-4 rel precision).
Host does only sharding/layout: slicing, transposes, index permutation, concat.
"""
import sys

for _p in ("/opt/trn_rl_repo",):
    if _p not in sys.path:
        sys.path.insert(0, _p)

import numpy as np
import concourse.bass as bass
import concourse.tile as tile
from concourse import bacc, mybir
from concourse.bass_utils import run_bass_kernel_spmd

from concourse.dve_spec import Spec, Src0, Src1, C0, C1, C2, One, select, lower
from concourse.dve_uop import DveOpSpec
from concourse import dve_ops
from concourse.dve_ops import DveOp, OPS


def _make_gsel2():
    """Custom DVE op: out = in0==imm2 ? s0 : (in0==imm2+1 ? s1 : in1).

    One instruction applies two codebook codes; 128 passes realize a full
    256-entry lookup at 1 elem/cycle/lane."""
    for op in OPS:
        if op.name == "GSEL2":
            return op
    import numpy as _np
    d = Src0 - C2
    spec = Spec(
        body=select(d * (d - One), Src1, C0 + d * (C1 - C0)),
        reference=lambda in0, in1, s0, s1, imm2: _np.where(
            in0 == imm2, s0, _np.where(in0 == imm2 + 1.0, s1, in1)
        ),
    )
    shas = {}
    for ver in ("v3", "v4"):
        shas[ver] = DveOpSpec(
            name="GSEL2", opcode=0, uops=lower(spec, ver=ver), rd1_en=True
        ).sha(ver)
    op = DveOp("GSEL2", spec, subdim=False, uops_sha=shas)
    OPS.append(op)
    dve_ops.CUSTOM_DVE_SPECS[op.name] = op.spec
    row = dve_ops._CUSTOM_DVE_ROW_BASE + len(OPS) - 1
    assert row < 0x20, "custom-DVE row field overflow"
    dve_ops._SUB_OPCODE_FOR_NAME[op.name] = row
    return op


GSEL2 = _make_gsel2()

F32, F32R = mybir.dt.float32, mybir.dt.float32r
I32, I16, U8 = mybir.dt.int32, mybir.dt.int16, mybir.dt.uint8
AF = mybir.ActivationFunctionType
OP = mybir.AluOpType

B, IN_D, H, OUT_D, CB = 8192, 2048, 8192, 2048, 256
NC = 8
H_S = H // NC            # 1024 H columns per core
BS = B // NC             # 1024 batch columns per core (decode shard)
NB = B // 512            # 16 batch blocks of 512
NPASS = CB // 2          # GSEL2 codebook passes (2 codes per DVE instruction)


def _build():
    nc = bacc.Bacc("TRN2", target_bir_lowering=False, debug=False, num_devices=NC)

    # ---- inputs (per core) ----
    xT = nc.dram_tensor("xT", [IN_D, B], F32R, kind="ExternalInput")
    idx1 = nc.dram_tensor("idx1", [IN_D, H_S], I32, kind="ExternalInput")
    idx2 = nc.dram_tensor("idx2", [H_S, OUT_D], I32, kind="ExternalInput")
    flt1 = nc.dram_tensor("flt1", [IN_D, H_S], F32, kind="ExternalInput")
    flt2 = nc.dram_tensor("flt2", [H_S, OUT_D], F32, kind="ExternalInput")
    msk1 = nc.dram_tensor("msk1", [IN_D, H_S], U8, kind="ExternalInput")
    msk2 = nc.dram_tensor("msk2", [H_S, OUT_D], U8, kind="ExternalInput")
    cb1 = nc.dram_tensor("cb1", [1, CB], F32, kind="ExternalInput")
    cb2 = nc.dram_tensor("cb2", [1, CB], F32, kind="ExternalInput")
    crw = nc.dram_tensor("crw", [128, 8], F32, kind="ExternalInput")
    rrw = nc.dram_tensor("rrw", [128, 8], F32, kind="ExternalInput")
    b1h = nc.dram_tensor("b1h", [128, 8], F32, kind="ExternalInput")
    db1h = nc.dram_tensor("db1h", [1, H_S], F32, kind="ExternalInput")
    b2h = nc.dram_tensor("b2h", [128, 16], F32, kind="ExternalInput")
    db2h = nc.dram_tensor("db2h", [128, 16], F32, kind="ExternalInput")

    # ---- outputs (per core) ----
    z_out = nc.dram_tensor("z_out", [OUT_D, BS], F32, kind="ExternalOutput")
    recon_out = nc.dram_tensor("recon_out", [IN_D, BS], F32, kind="ExternalOutput")

    with tile.TileContext(nc) as tc:
        with (
            tc.tile_pool(name="params", bufs=1) as params,
            tc.tile_pool(name="dram", bufs=1, space="DRAM") as dram,
        ):
            z_stage = dram.tile([NC, OUT_D, BS], F32, tag="zst")
            zrs = dram.tile([OUT_D, BS], F32, tag="zrs")
            m_stage = dram.tile([OUT_D + 1, IN_D], F32, tag="mst")
            m_ar = dram.tile([OUT_D + 1, IN_D], F32, tag="mar")

            wctx = tc.tile_pool(name="w1pool", bufs=1)
            w1pool = wctx.__enter__()
            wctx2 = tc.tile_pool(name="w2pool", bufs=1)
            w2pool = wctx2.__enter__()
            # ---- construct W tiles: GSEL2 codebook passes + predicated select ----
            # G starts as the float weights; 128 GSEL2 passes overwrite frozen
            # positions (mask folded into the index: unfrozen -> 999, no match).
            w1t = []
            w2t = []
            with nc.named_scope("construct"):
                db1s = params.tile([128, H_S], F32, tag="db1s")
                nc.sync.dma_start(db1s[:], db1h.ap().to_broadcast((128, H_S)))
                cb1s = params.tile([128, CB], F32, tag="cb1s")
                nc.sync.dma_start(cb1s[:], cb1.ap().to_broadcast((128, CB)))
                cb2s = params.tile([128, CB], F32, tag="cb2s")
                nc.sync.dma_start(cb2s[:], cb2.ap().to_broadcast((128, CB)))
                with tc.tile_pool(name="cpool", bufs=2) as cpool, \
                        tc.tile_pool(name="gacc", bufs=1) as gacc:
                    # regions: W1 as 8 chunk-pairs, W2 as 8 single chunks.
                    # Every region is a [128, 2048] tile; W1 pair r holds
                    # chunks 2r (cols 0:1024) and 2r+1 (cols 1024:2048).
                    regions = []
                    for r in range(8):
                        regions.append((0, r))
                    for k in range(8):
                        regions.append((1, k))
                    for wi, r in regions:
                        cbs = cb1s if wi == 0 else cb2s
                        i32t = cpool.tile([128, 2048], I32, tag="i32")
                        fl = cpool.tile([128, 2048], F32, tag="fl")
                        mk = cpool.tile([128, 2048], U8, tag="mk")
                        if wi == 0:
                            for h in range(2):
                                k = 2 * r + h
                                sl = slice(h * H_S, (h + 1) * H_S)
                                nc.sync.dma_start(
                                    i32t[:, sl], idx1[k * 128:(k + 1) * 128, :])
                                nc.sync.dma_start(
                                    fl[:, sl], flt1[k * 128:(k + 1) * 128, :])
                                nc.sync.dma_start(
                                    mk[:, sl], msk1[k * 128:(k + 1) * 128, :])
                        else:
                            nc.sync.dma_start(
                                i32t[:], idx2[r * 128:(r + 1) * 128, :])
                            nc.sync.dma_start(
                                fl[:], flt2[r * 128:(r + 1) * 128, :])
                            nc.sync.dma_start(
                                mk[:], msk2[r * 128:(r + 1) * 128, :])
                        idxm = gacc.tile([128, 2048], F32, tag="idxm")
                        nc.vector.memset(idxm[:], 999.0)
                        nc.vector.copy_predicated(idxm[:], mk[:], i32t[:])
                        G = gacc.tile([128, 2048], F32, tag="G")
                        nc.vector.tensor_copy(G[:], fl[:])
                        for j in range(NPASS):
                            nc.vector._custom_dve(
                                GSEL2, out=G[:], in0=idxm[:], in1=G[:],
                                s0=cbs[:, 2 * j:2 * j + 1],
                                s1=cbs[:, 2 * j + 1:2 * j + 2],
                                imm2=float(2 * j),
                            )
                        if wi == 0:
                            for h in range(2):
                                k = 2 * r + h
                                sl = slice(h * H_S, (h + 1) * H_S)
                                wk = w1pool.tile([128, H_S], F32R, tag=f"w1_{k}")
                                nc.scalar.copy(wk[:], G[:, sl])
                                w1t.append(wk)
                                vdum = cpool.tile([128, H_S], F32, tag="vdum")
                                vk = cpool.tile([128, 1], F32, tag="vk")
                                nc.vector.scalar_tensor_tensor(
                                    vdum[:], G[:, sl], 1.0, db1s[:],
                                    OP.mult, OP.mult, accum_out=vk[:],
                                )
                                nc.sync.dma_start(
                                    m_stage[
                                        OUT_D:OUT_D + 1, k * 128:(k + 1) * 128
                                    ].rearrange("a b -> (a b)").rearrange(
                                        "(a b) -> a b", b=1
                                    ),
                                    vk[:],
                                )
                        else:
                            wk = w2pool.tile([128, OUT_D], F32R, tag=f"w2_{r}")
                            nc.scalar.copy(wk[:], G[:])
                            w2t.append(wk)

            # ---------- c19 per-partition params ([128, 8]) ----------
            with nc.named_scope("params"):
                craw = params.tile([128, 8], F32, tag="craw")
                rraw = params.tile([128, 8], F32, tag="rraw")
                b1s = params.tile([128, 8], F32, tag="b1s")
                b2s = params.tile([128, 16], F32, tag="b2s")
                db2s = params.tile([128, 16], F32, tag="db2s")
                nc.sync.dma_start(craw[:], crw.ap())
                nc.sync.dma_start(rraw[:], rrw.ap())
                nc.sync.dma_start(b1s[:], b1h.ap())
                nc.sync.dma_start(b2s[:], b2h.ap())
                nc.sync.dma_start(db2s[:], db2h.ap())
                c_sb = params.tile([128, 8], F32, tag="c")
                rho = params.tile([128, 8], F32, tag="rho")
                inv_c = params.tile([128, 8], F32, tag="invc")
                s1 = params.tile([128, 8], F32, tag="s1")
                s2 = params.tile([128, 8], F32, tag="s2")
                b1c = params.tile([128, 8], F32, tag="b1c")
                tmp8 = params.tile([128, 8], F32, tag="tmp8")
                exp_c = params.tile([128, 8], F32, tag="expc")
                nc.scalar.activation(exp_c[:], craw[:], AF.Exp)
                nc.scalar.activation(c_sb[:], exp_c[:], AF.Ln, bias=1.0)
                nc.scalar.activation(rho[:], rraw[:], AF.Sigmoid)
                nc.vector.reciprocal(inv_c[:], c_sb[:])
                nc.vector.tensor_scalar(tmp8[:], rho[:], -1.0, 1.0, OP.mult, OP.add)
                nc.vector.tensor_tensor(s1[:], tmp8[:], c_sb[:], OP.mult)
                nc.vector.tensor_tensor(s2[:], rho[:], b1s[:], OP.mult)
                nc.vector.tensor_tensor(b1c[:], b1s[:], inv_c[:], OP.mult)
                ones = params.tile([128, 128], F32, tag="ones")
                nc.vector.memset(ones[:], 1.0)
                ident_f = params.tile([128, 128], F32, tag="identf")
                nc.gpsimd.affine_select(
                    ident_f[:], ones[:], pattern=[[-1, 128]],
                    compare_op=OP.is_equal, fill=0.0, base=0, channel_multiplier=1,
                )
                ident = params.tile([128, 128], F32R, tag="ident")
                nc.scalar.copy(ident[:], ident_f[:])

            # ---------- encode: 16 batch blocks of 512 ----------
            with nc.named_scope("encode"):
                with (
                    tc.tile_pool(name="xpool", bufs=2) as xpool,
                    tc.tile_pool(name="hpool", bufs=10) as hpool,
                    tc.tile_pool(name="tpool", bufs=3) as tpool,
                    tc.tile_pool(name="p1", bufs=3, space="PSUM") as p1,
                    tc.tile_pool(name="p2", bufs=2, space="PSUM") as p2,
                    tc.tile_pool(name="zpool", bufs=3) as zpool,
                ):
                    for n in range(NB):
                        xh = []
                        for half in range(2):
                            xt = xpool.tile([128, 8 * 512], F32R, tag="x")
                            src = xT[
                                half * 1024:(half + 1) * 1024,
                                n * 512:(n + 1) * 512,
                            ].rearrange("(c p) j -> p c j", p=128)
                            nc.sync.dma_start(
                                xt.rearrange("p (c j) -> p c j", j=512), src
                            )
                            xh.append(xt)
                        h_tiles = []
                        for m in range(8):
                            ps = p1.tile([128, 512], F32, tag="ps1")
                            for k in range(16):
                                nc.tensor.matmul(
                                    ps[:],
                                    w1t[k][:, m * 128:(m + 1) * 128],
                                    xh[k // 8][:, (k % 8) * 512:(k % 8 + 1) * 512],
                                    start=(k == 0), stop=(k == 15),
                                )
                            t_t = tpool.tile([128, 512], F32, tag="t")
                            nc.scalar.activation(
                                t_t[:], ps[:], AF.Tanh,
                                bias=b1c[:, m:m + 1], scale=inv_c[:, m:m + 1],
                            )
                            nc.vector.tensor_scalar(
                                t_t[:], t_t[:], s1[:, m:m + 1], s2[:, m:m + 1],
                                OP.mult, OP.add,
                            )
                            h_m = hpool.tile([128, 512], F32R, tag="h")
                            nc.vector.scalar_tensor_tensor(
                                h_m[:], ps[:], rho[:, m:m + 1], t_t[:],
                                OP.mult, OP.add,
                            )
                            h_tiles.append(h_m)
                        for mo in range(16):
                            ps2 = p2.tile([128, 512], F32, tag="ps2")
                            for c in range(8):
                                nc.tensor.matmul(
                                    ps2[:],
                                    w2t[c][:, mo * 128:(mo + 1) * 128],
                                    h_tiles[c][:],
                                    start=(c == 0), stop=(c == 7),
                                )
                            zt = zpool.tile([128, 512], F32, tag="z")
                            nc.scalar.copy(zt[:], ps2[:])
                            nc.sync.dma_start(
                                z_stage[
                                    n // 2,
                                    mo * 128:(mo + 1) * 128,
                                    (n % 2) * 512:(n % 2 + 1) * 512,
                                ],
                                zt[:],
                            )

            # ---------- ReduceScatter z over batch ----------
            with nc.named_scope("rs"):
                nc.gpsimd.collective_compute(
                    "ReduceScatter", OP.add,
                    replica_groups=[list(range(NC))],
                    ins=[z_stage.opt()], outs=[zrs.opt()],
                )

            # ---------- M = W2_eff.T @ W1_eff.T partial + v = W1@db1 ----------
            with nc.named_scope("mbuild"):
                with (
                    tc.tile_pool(name="w1tp", bufs=1) as w1tp,
                    tc.tile_pool(name="pt", bufs=2, space="PSUM") as pt,
                    tc.tile_pool(name="pm", bufs=2, space="PSUM") as pm,
                    tc.tile_pool(name="mout", bufs=3) as mout,
                ):
                    for half in range(2):
                        # transpose W1_s chunks of this in-half -> W1T half tiles
                        ht_tiles = []
                        for c in range(8):
                            w1tc = w1tp.tile([128, 1024], F32R, tag=f"w1t_{c}")
                            for kk in range(8):
                                k = half * 8 + kk
                                pst = pt.tile([128, 128], F32R, tag="pst")
                                nc.tensor.transpose(
                                    pst[:],
                                    w1t[k][:, c * 128:(c + 1) * 128],
                                    ident[:],
                                )
                                nc.scalar.copy(
                                    w1tc[:, kk * 128:(kk + 1) * 128], pst[:]
                                )
                            ht_tiles.append(w1tc)
                        for mo in range(16):
                            for nin in range(2):
                                psm = pm.tile([128, 512], F32, tag="psm")
                                for c in range(8):
                                    nc.tensor.matmul(
                                        psm[:],
                                        w2t[c][:, mo * 128:(mo + 1) * 128],
                                        ht_tiles[c][:, nin * 512:(nin + 1) * 512],
                                        start=(c == 0), stop=(c == 7),
                                    )
                                ms = mout.tile([128, 512], F32, tag="ms")
                                nc.scalar.copy(ms[:], psm[:])
                                nc.sync.dma_start(
                                    m_stage[
                                        mo * 128:(mo + 1) * 128,
                                        half * 1024 + nin * 512:
                                        half * 1024 + (nin + 1) * 512,
                                    ],
                                    ms[:],
                                )

            with nc.named_scope("arm"):
                nc.gpsimd.collective_compute(
                    "AllReduce", OP.add,
                    replica_groups=[list(range(NC))],
                    ins=[m_stage.opt()], outs=[m_ar.opt()],
                )

            wctx2.__exit__(None, None, None)
            wctx.__exit__(None, None, None)

            # ---------- decode: recon.T shard = sum_k M[k,:].T @ z.T[k,:] ----------
            with nc.named_scope("decode"):
                with (
                    tc.tile_pool(name="mpool", bufs=1) as mpool,
                    tc.tile_pool(name="zq", bufs=3) as zq,
                    tc.tile_pool(name="zr", bufs=17) as zr,
                    tc.tile_pool(name="p3", bufs=3, space="PSUM") as p3,
                    tc.tile_pool(name="ro", bufs=3) as ro,
                ):
                    vd = params.tile([128, 16], F32, tag="vd")
                    nc.sync.dma_start(
                        vd[:],
                        m_ar[OUT_D:OUT_D + 1, :].rearrange(
                            "one (m p) -> (one p) m", p=128
                        ),
                    )
                    nc.vector.tensor_tensor(vd[:], vd[:], db2s[:], OP.add)
                    m_tiles = []
                    for k in range(16):
                        mt = mpool.tile([128, IN_D], F32R, tag=f"m_{k}")
                        nc.sync.dma_start(
                            mt[:], m_ar[k * 128:(k + 1) * 128, :].bitcast(F32R)
                        )
                        m_tiles.append(mt)
                    for n in range(2):
                        zr_tiles = []
                        for k in range(16):
                            zt = zq.tile([128, 512], F32, tag="zq")
                            nc.sync.dma_start(
                                zt[:],
                                zrs[k * 128:(k + 1) * 128, n * 512:(n + 1) * 512],
                            )
                            nc.vector.tensor_scalar(
                                zt[:], zt[:], b2s[:, k:k + 1], None, OP.add
                            )
                            nc.sync.dma_start(
                                z_out[k * 128:(k + 1) * 128, n * 512:(n + 1) * 512],
                                zt[:],
                            )
                            zk = zr.tile([128, 512], F32R, tag="zr")
                            nc.scalar.copy(zk[:], zt[:])
                            zr_tiles.append(zk)
                        for m in range(16):
                            ps3 = p3.tile([128, 512], F32, tag="ps3")
                            for k in range(16):
                                nc.tensor.matmul(
                                    ps3[:],
                                    m_tiles[k][:, m * 128:(m + 1) * 128],
                                    zr_tiles[k][:],
                                    start=(k == 0), stop=(k == 15),
                                )
                            rt = ro.tile([128, 512], F32, tag="ro")
                            nc.vector.tensor_scalar(
                                rt[:], ps3[:], vd[:, m:m + 1], None, OP.add
                            )
                            nc.sync.dma_start(
                                recon_out[
                                    m * 128:(m + 1) * 128, n * 512:(n + 1) * 512
                                ],
                                rt[:],
                            )

    nc.compile()
    return nc


_CACHE = {}


def _get_nc():
    if "nc" not in _CACHE:
        _CACHE["nc"] = _build()
    return _CACHE["nc"]


def _prep_in_maps(inputs):
    x = np.asarray(inputs["x"], np.float32)
    xT = np.ascontiguousarray(x.T)
    cb1 = np.asarray(inputs["codebook_W1"], np.float32).reshape(1, CB)
    cb2 = np.asarray(inputs["codebook_W2"], np.float32).reshape(1, CB)
    W1f, W2f = np.asarray(inputs["W1_float"], np.float32), np.asarray(inputs["W2_float"], np.float32)
    W1i, W2i = np.asarray(inputs["W1_idx"], np.int32), np.asarray(inputs["W2_idx"], np.int32)
    W1m = np.asarray(inputs["W1_frozen_mask"]).astype(np.uint8)
    W2m = np.asarray(inputs["W2_frozen_mask"]).astype(np.uint8)
    b1 = np.asarray(inputs["b1"], np.float32)
    b2 = np.asarray(inputs["b2"], np.float32)
    db1 = np.asarray(inputs["db1"], np.float32)
    db2 = np.asarray(inputs["db2"], np.float32)
    craw = np.asarray(inputs["c19_c_raw"], np.float32)
    rraw = np.asarray(inputs["c19_rho_raw"], np.float32)

    def p8(v):   # [1024] -> [128, 8]
        return np.ascontiguousarray(v.reshape(8, 128).T)

    def p16(v):  # [2048] -> [128, 16]
        return np.ascontiguousarray(v.reshape(16, 128).T)

    in_maps = []
    for c in range(NC):
        hs = slice(H_S * c, H_S * (c + 1))
        in_maps.append(dict(
            xT=xT,
            idx1=np.ascontiguousarray(W1i[:, hs]),
            idx2=np.ascontiguousarray(W2i[hs, :]),
            flt1=np.ascontiguousarray(W1f[:, hs]),
            flt2=np.ascontiguousarray(W2f[hs, :]),
            msk1=np.ascontiguousarray(W1m[:, hs]),
            msk2=np.ascontiguousarray(W2m[hs, :]),
            cb1=cb1, cb2=cb2,
            crw=p8(craw[hs]), rrw=p8(rraw[hs]), b1h=p8(b1[hs]),
            db1h=np.ascontiguousarray(db1[hs]).reshape(1, H_S), b2h=p16(b2), db2h=p16(db2),
        ))
    return in_maps


def _assemble(results):
    reconT = np.concatenate([results[c]["recon_out"] for c in range(NC)], axis=1)
    zT = np.concatenate([results[c]["z_out"] for c in range(NC)], axis=1)
    recon = np.ascontiguousarray(reconT.T, dtype=np.float32)
    z = np.ascontiguousarray(zT.T, dtype=np.float32)
    return recon, z


def kernel(**inputs):
    nc = _get_nc()
    in_maps = _prep_in_maps(inputs)
    res = run_bass_kernel_spmd(nc, in_maps, core_ids=list(range(NC)))
    return _assemble(res.results)
